# revision 10
# baseline (speedup 1.0000x reference)
"""nn_AutoregBlock forward on 8 trn2 NeuronCores (data-parallel over batch).

Self-contained: inline graph construction (bass/tile), host-side input
arrangement, device execution via run_bass_kernel_spmd, output reassembly.
Falls back to a vectorized numpy implementation if the device path fails.
"""
import os
import sys
import numpy as np

B, DIM, HID, K, G, MID = 1024, 384, 64, 8, 48, 128
EPS = 1e-6
NCORES = 8
BL = B // NCORES
R = 16
NBLK = BL // R
NPAIR = R // 2
NCH = DIM // 128


# --------------------------------------------------------------------------
# graph construction (see kernel_build.py history for design notes)
# --------------------------------------------------------------------------
def _legalize_waits(nc, max_waits=1):
    import concourse.mybir as mybir
    total = 0
    for fn in nc.m.functions:
        for bb in fn.blocks:
            new_insts = []
            for inst in bb.instructions:
                si = inst.sync_info
                waits = list(si.on_wait) if si and si.on_wait else []
                if len(waits) > max_waits:
                    extra, keep = waits[:-max_waits], waits[-max_waits:]
                    for k, w in enumerate(extra):
                        nop = mybir.InstNoOp(name=f"{inst.name}_lw{k}")
                        nop.engine = inst.engine
                        nop.sync_info = mybir.SyncInfo(on_wait=[w], on_update=[])
                        new_insts.append(nop)
                        total += 1
                    si.on_wait = keep
                new_insts.append(inst)
            bb.instructions = new_insts
    return total


def _build_graph():
    import concourse.bass as bass
    import concourse.mybir as mybir
    from concourse.tile import TileContext

    f32 = mybir.dt.float32
    bf16 = mybir.dt.bfloat16
    AF = mybir.ActivationFunctionType
    ALU = mybir.AluOpType
    AX = mybir.AxisListType

    nc = bass.Bass()
    dp = nc.declare_dram_parameter
    # x^T / inv^T, x8-replicated, laid out [c][blk][d][r][i] so the DMA
    # source merges (d, r) into one stride-8 run (3-dim DMA AP cap)
    sT = dp("sT", [NCH * NBLK * 128, R * 8], bf16, isOutput=False)
    iT = dp("iT", [NCH * NBLK * 128, R * 8], bf16, isOutput=False)
    Wd = dp("Wd", [DIM, HID], bf16, isOutput=False)
    Bd = dp("Bd", [DIM, HID], bf16, isOutput=False)
    Mw = dp("Mw", [128, 128], bf16, isOutput=False)
    I1 = dp("I1", [128, 128], bf16, isOutput=False)
    pw1 = dp("pw1", [128, G], f32, isOutput=False)
    pb1 = dp("pb1", [128, G], f32, isOutput=False)
    pw2 = dp("pw2", [128, G], f32, isOutput=False)
    pb2 = dp("pb2", [128, G], f32, isOutput=False)
    pjt = dp("pjt", [128, NCH * 128], bf16, isOutput=False)
    pjb = dp("pjb", [128, NCH], f32, isOutput=False)
    W1s = dp("W1s", [128, MID], bf16, isOutput=False)
    dbv = dp("dbv", [MID, 1], f32, isOutput=False)
    W2s = dp("W2s", [MID, HID], bf16, isOutput=False)
    ubv = dp("ubv", [128, 1], f32, isOutput=False)
    out = dp("out", [128, (BL // 2) * DIM], f32, isOutput=True)

    with TileContext(nc) as tc:
        with tc.tile_pool(name="const", bufs=1) as cp, \
             tc.tile_pool(name="io", bufs=2) as io, \
             tc.tile_pool(name="ctp", bufs=4) as ctpool, \
             tc.tile_pool(name="htp", bufs=64) as htpool, \
             tc.tile_pool(name="pgsp", bufs=NBLK) as pgpool, \
             tc.tile_pool(name="dram", bufs=2, space="DRAM") as dr:
            w_rep = cp.tile([128, NCH, R, HID], bf16)
            b_rep = cp.tile([128, NCH, R, HID], bf16)
            for c in range(NCH):
                nc.sync.dma_start(
                    out=w_rep[:, c], in_=Wd[128 * c:128 * (c + 1), None, :]
                    .to_broadcast((128, R, HID)))
                nc.sync.dma_start(
                    out=b_rep[:, c], in_=Bd[128 * c:128 * (c + 1), None, :]
                    .to_broadcast((128, R, HID)))
            mwt = cp.tile([128, 128], bf16)
            nc.sync.dma_start(out=mwt[:], in_=Mw[:])
            i1t = cp.tile([128, 128], bf16)
            nc.sync.dma_start(out=i1t[:], in_=I1[:])
            pw1t = cp.tile([128, G], f32)
            nc.sync.dma_start(out=pw1t[:], in_=pw1[:])
            pb1t = cp.tile([128, G], f32)
            nc.sync.dma_start(out=pb1t[:], in_=pb1[:])
            pw2t = cp.tile([128, G], f32)
            nc.sync.dma_start(out=pw2t[:], in_=pw2[:])
            pb2t = cp.tile([128, G], f32)
            nc.sync.dma_start(out=pb2t[:], in_=pb2[:])
            pjtt = cp.tile([128, NCH * 128], bf16)
            nc.sync.dma_start(out=pjtt[:], in_=pjt[:])
            pjbt = cp.tile([128, NCH], f32)
            nc.sync.dma_start(out=pjbt[:], in_=pjb[:])
            w1t = cp.tile([128, MID], bf16)
            nc.sync.dma_start(out=w1t[:], in_=W1s[:])
            dbt = cp.tile([MID, 1], f32)
            nc.sync.dma_start(out=dbt[:], in_=dbv[:])
            w2t = cp.tile([MID, HID], bf16)
            nc.sync.dma_start(out=w2t[:], in_=W2s[:])
            ubt = cp.tile([128, 1], f32)
            nc.sync.dma_start(out=ubt[:], in_=ubv[:])

            hts = []
            pgs = []

            # phase A1: features, Ct/Ht, Gram
            with tc.tile_pool(name="psA", bufs=2, space="PSUM") as pp:
                for blk in range(NBLK):
                    r0 = blk * R
                    s_rep = io.tile([128, NCH, R, 8, 8], bf16, tag="s_rep")
                    i_rep = io.tile([128, NCH, R, 8, 8], bf16, tag="i_rep")
                    for c in range(NCH):
                        row = (c * NBLK + blk) * 128
                        nc.sync.dma_start(
                            out=s_rep[:, c],
                            in_=sT[row:row + 128, :]
                            .rearrange("p (r i) -> p r i", i=8)[:, :, None, :]
                            .to_broadcast((128, R, 8, 8)))
                        nc.sync.dma_start(
                            out=i_rep[:, c],
                            in_=iT[row:row + 128, :]
                            .rearrange("p (r i) -> p r i", i=8)[:, :, None, :]
                            .to_broadcast((128, R, 8, 8)))
                    hd = io.tile([128, NCH, R, HID], bf16, tag="hd")
                    for c in range(NCH):
                        t1 = io.tile([128, R, HID], bf16, tag="t1")
                        nc.vector.tensor_tensor(
                            out=t1[:], in0=w_rep[:, c],
                            in1=s_rep[:, c].rearrange("p r a b -> p r (a b)"),
                            op=ALU.mult)
                        t2 = io.tile([128, R, HID], bf16, tag="t2")
                        nc.vector.tensor_tensor(
                            out=t2[:], in0=b_rep[:, c],
                            in1=i_rep[:, c].rearrange("p r a b -> p r (a b)"),
                            op=ALU.mult)
                        av = io.tile([128, R, HID], bf16, tag="av")
                        nc.vector.tensor_tensor(out=av[:], in0=t1[:], in1=t2[:],
                                                op=ALU.add)
                        nc.scalar.activation(hd[:, c], av[:], AF.Gelu)

                    pgp = pp.tile([128, 512], f32, tag="pg")
                    for p in range(NPAIR):
                        ctp = pp.tile([128, 512], f32, tag="ctp")
                        htp = pp.tile([128, 512], f32, tag="htp")
                        for c in range(NCH):
                            lhs = hd[:, c, 2 * p:2 * p + 2, :]
                            nc.tensor.matmul(ctp[:, 128 * c:128 * (c + 1)],
                                             lhs, mwt[:], start=True, stop=True)
                            nc.tensor.matmul(htp[:, 128 * c:128 * (c + 1)],
                                             lhs, i1t[:], start=True, stop=True)
                        ct = ctpool.tile([128, DIM], bf16, tag="ct")
                        nc.vector.tensor_copy(ct[:], ctp[:, 0:DIM])
                        ht = htpool.tile([128, DIM], bf16, tag="ht")
                        nc.scalar.copy(ht[:], htp[:, 0:DIM])
                        hts.append(ht)
                        ctv = ct[:].rearrange("p (g k) -> p k g", k=K)
                        htv = ht[:].rearrange("p (g k) -> p k g", k=K)
                        for l in range(K):
                            nc.tensor.matmul(
                                pgp[0:G, G * p:G * (p + 1)],
                                ctv[0:64, l], htv[0:64, l],
                                start=(l == 0), stop=(l == K - 1))
                        for l in range(K):
                            nc.tensor.matmul(
                                pgp[64:64 + G, G * p:G * (p + 1)],
                                ctv[64:128, l], htv[64:128, l],
                                start=(l == 0), stop=(l == K - 1),
                                tile_position=(64, 64))
                    pgt = pgpool.tile([112, NPAIR * G], f32, tag="pgs")
                    nc.vector.tensor_copy(pgt[:], pgp[0:112, 0:NPAIR * G])
                    pgs.append(pgt)

            # phase A2: gates, softmax, row weights, scales, H *= scales
            with tc.tile_pool(name="psS", bufs=2, space="PSUM") as sp:
                for blk in range(NBLK):
                    g1 = pgs[blk][:].rearrange("p (n g) -> p n g", g=G)
                    w1b = pw1t[0:112, None, :].to_broadcast((112, NPAIR, G))
                    b1b = pb1t[0:112, None, :].to_broadcast((112, NPAIR, G))
                    w2b = pw2t[0:112, None, :].to_broadcast((112, NPAIR, G))
                    b2b = pb2t[0:112, None, :].to_broadcast((112, NPAIR, G))
                    nc.vector.tensor_tensor(out=g1, in0=g1, in1=w1b, op=ALU.mult)
                    nc.vector.tensor_tensor(out=g1, in0=g1, in1=b1b, op=ALU.add)
                    nc.scalar.activation(g1, g1, AF.Lrelu, alpha=0.01)
                    nc.vector.tensor_tensor(out=g1, in0=g1, in1=w2b, op=ALU.mult)
                    nc.vector.tensor_tensor(out=g1, in0=g1, in1=b2b, op=ALU.add)
                    nc.scalar.activation(g1, g1, AF.Lrelu, alpha=0.01)
                    mx = io.tile([112, NPAIR], f32, tag="mx")
                    nc.vector.tensor_reduce(mx[:], g1, axis=AX.X, op=ALU.max)
                    nc.vector.tensor_tensor(
                        out=g1, in0=g1,
                        in1=mx[:, :, None].to_broadcast((112, NPAIR, G)),
                        op=ALU.subtract)
                    nc.scalar.activation(g1, g1, AF.Exp)
                    sm = io.tile([112, NPAIR], f32, tag="sm")
                    nc.vector.tensor_reduce(sm[:], g1, axis=AX.X, op=ALU.add)
                    si = io.tile([112, NPAIR], f32, tag="si")
                    nc.vector.reciprocal(si[:], sm[:])
                    si2 = io.tile([112, NPAIR], f32, tag="si2")
                    nc.vector.tensor_tensor(out=si2[:], in0=si[:], in1=si[:],
                                            op=ALU.mult)
                    nc.scalar.activation(g1, g1, AF.Square)
                    nc.vector.tensor_tensor(
                        out=g1, in0=g1,
                        in1=si2[:, :, None].to_broadcast((112, NPAIR, G)),
                        op=ALU.mult)
                    nc.scalar.activation(g1, g1, AF.Ln, bias=1.0)
                    nc.scalar.activation(g1, g1, AF.Exp, scale=-1.0)
                    rw = io.tile([112, NPAIR], f32, tag="rw")
                    nc.vector.tensor_reduce(rw[:], g1, axis=AX.X, op=ALU.add)
                    rwb = io.tile([112, NPAIR], bf16, tag="rwb")
                    nc.vector.tensor_scalar_add(rwb[:], rw[:], -47.0)

                    scp = sp.tile([128, 512], f32, tag="scp")
                    scpv = scp[:, 0:NCH * 2 * NPAIR].rearrange(
                        "p (c r n) -> p c r n", c=NCH, r=2)
                    for c in range(NCH):
                        nc.tensor.matmul(scpv[:, c, 0],
                                         pjtt[0:G, 128 * c:128 * (c + 1)],
                                         rwb[0:G, :], start=True, stop=True)
                        nc.tensor.matmul(scpv[:, c, 1],
                                         pjtt[64:64 + G, 128 * c:128 * (c + 1)],
                                         rwb[64:64 + G, :], start=True,
                                         stop=True, tile_position=(64, 0))
                    scs = io.tile([128, 2, NCH, NPAIR], bf16, tag="scs")
                    for c in range(NCH):
                        nc.scalar.activation(scs[:, :, c, :], scpv[:, c],
                                             AF.Identity, bias=pjbt[:, c:c + 1])
                    dsc = dr.tile([2, NPAIR, NCH, 128], bf16, tag="dsc")
                    scb = io.tile([128, NPAIR, DIM], bf16, tag="scb")
                    for rh in range(2):
                        for c in range(NCH):
                            nc.sync.dma_start(
                                out=dsc[rh, :, c, :].rearrange("p d -> d p"),
                                in_=scs[:, rh, c, :])
                        nc.sync.dma_start(
                            out=scb[64 * rh:64 * (rh + 1)],
                            in_=dsc[rh].rearrange("p c d -> (p c d)")
                            [None].to_broadcast((64, NPAIR * DIM))
                            .rearrange("h (p d) -> h p d", p=NPAIR))
                    for p in range(NPAIR):
                        ht = hts[blk * NPAIR + p]
                        nc.vector.tensor_tensor(out=ht[:], in0=ht[:],
                                                in1=scb[:, p, :], op=ALU.mult)

            # phase B: MLP
            with tc.tile_pool(name="psB", bufs=2, space="PSUM") as bp:
                for p2 in range(BL // 4):
                    z2p = bp.tile([128, 2, 512], f32, tag="z2p")
                    for pi in range(2):
                        hs = hts[2 * p2 + pi]
                        z1p = bp.tile([MID, 2, 512], f32, tag="z1p")
                        nc.tensor.matmul(z1p[:, 0, 0:DIM], w1t[0:64, :],
                                         hs[0:64, :], start=True, stop=True)
                        nc.tensor.matmul(z1p[:, 1, 0:DIM], w1t[64:128, :],
                                         hs[64:128, :], start=True,
                                         stop=True, tile_position=(64, 0))
                        z1s = io.tile([MID, 2, DIM], bf16, tag="z1s")
                        nc.scalar.activation(z1s[:], z1p[:, :, 0:DIM], AF.Lrelu,
                                             bias=dbt[:], alpha=0.01)
                        nc.tensor.matmul(z2p[0:64, pi, 0:DIM], w2t[:],
                                         z1s[:, 0], start=True, stop=True)
                        nc.tensor.matmul(z2p[64:128, pi, 0:DIM], w2t[:],
                                         z1s[:, 1], start=True, stop=True,
                                         tile_position=(0, 64))
                    z2s = io.tile([128, 2, DIM], f32, tag="z2s")
                    nc.scalar.activation(z2s[:], z2p[:, :, 0:DIM], AF.Lrelu,
                                         bias=ubt[:], alpha=0.01)
                    nc.sync.dma_start(
                        out=out[:, (2 * p2) * DIM:(2 * p2 + 2) * DIM],
                        in_=z2s[:])
    _legalize_waits(nc)
    return nc


def _host_prepare(x, W, b, patch_weight, pair_w1, pair_b1, pair_w2, pair_b2,
                  proj_w, proj_b, down_w, down_b, up_w, up_b):
    import ml_dtypes
    bf = ml_dtypes.bfloat16
    x = np.asarray(x, np.float32)
    inv = 1.0 / (np.abs(x) + np.float32(EPS))
    pw = np.asarray(patch_weight, np.float32).reshape(-1)
    e = np.exp(pw - pw.max())
    w = (e / e.sum()).reshape(K, K)
    mw = np.zeros((128, 128), np.float32)
    for g in range(16):
        mw[g * K:(g + 1) * K, g * K:(g + 1) * K] = w
    stack = lambda a: np.concatenate([a, a], axis=0)

    def gate(m):
        t = np.zeros((128, G), np.float32)
        t[0:G] = m
        t[64:64 + G] = m
        return t

    pjt = np.zeros((128, NCH * 128), np.float32)
    for c in range(NCH):
        blkw = np.asarray(proj_w, np.float32)[128 * c:128 * (c + 1), :].T
        pjt[0:G, 128 * c:128 * (c + 1)] = blkw
        pjt[64:64 + G, 128 * c:128 * (c + 1)] = blkw
    pjb_full = (np.asarray(proj_b, np.float32)
                + 47.0 * np.asarray(proj_w, np.float32).sum(axis=1))
    pjb = pjb_full.reshape(NCH, 128).T.copy()
    w1s = stack(np.asarray(down_w, np.float32).T)
    w2s = np.ascontiguousarray(np.asarray(up_w, np.float32).T)
    ub2 = stack(np.asarray(up_b, np.float32).reshape(HID, 1))

    shared = {
        "Wd": np.asarray(W, np.float32).astype(bf),
        "Bd": np.asarray(b, np.float32).astype(bf),
        "Mw": mw.astype(bf),
        "I1": np.eye(128, dtype=np.float32).astype(bf),
        "pw1": gate(np.asarray(pair_w1, np.float32)),
        "pb1": gate(np.asarray(pair_b1, np.float32)),
        "pw2": gate(np.asarray(pair_w2, np.float32)),
        "pb2": gate(np.asarray(pair_b2, np.float32)),
        "pjt": pjt.astype(bf),
        "pjb": np.ascontiguousarray(pjb),
        "W1s": w1s.astype(bf),
        "dbv": np.asarray(down_b, np.float32).reshape(MID, 1),
        "W2s": w2s.astype(bf),
        "ubv": ub2,
    }
    def arrange_rep(a):
        # (BL, DIM) slice -> [c][blk][d][r][i] with x8 replication on i
        t = a.T.reshape(NCH, 128, NBLK, R)            # (c, d, blk, r)
        t = np.repeat(t.transpose(0, 2, 1, 3)[..., None], 8, axis=4)
        return np.ascontiguousarray(
            t.reshape(NCH * NBLK * 128, R * 8)).astype(bf)

    in_maps = []
    for c in range(NCORES):
        m = dict(shared)
        m["sT"] = arrange_rep(x[c * BL:(c + 1) * BL])
        m["iT"] = arrange_rep(inv[c * BL:(c + 1) * BL])
        in_maps.append(m)
    return in_maps


def _assemble_output(results):
    z = np.empty((B, DIM, HID), np.float32)
    for c in range(NCORES):
        arr = results[c]["out"].reshape(2, 64, BL // 2, DIM)
        z[c * BL:(c + 1) * BL] = arr.transpose(2, 0, 3, 1).reshape(BL, DIM, HID)
    return z


_CACHE = {}


def _device_run(in_maps):
    from concourse.bass_utils import run_bass_kernel_spmd
    if "nc" not in _CACHE:
        _CACHE["nc"] = _build_graph()
    res = run_bass_kernel_spmd(_CACHE["nc"], in_maps, list(range(NCORES)))
    return res.results


def _cpu_fallback(x, W, b, patch_weight, pair_w1, pair_b1, pair_w2, pair_b2,
                  proj_w, proj_b, down_w, down_b, up_w, up_b):
    from scipy.special import erf
    x = np.asarray(x, np.float32)
    s = x[:, :, None]
    inv = 1.0 / (np.abs(s) + np.float32(EPS))
    A = s * np.asarray(W, np.float32)[None] + inv * np.asarray(b, np.float32)[None]
    H = 0.5 * A * (1.0 + erf(A / np.sqrt(2.0, dtype=np.float32)))
    pw = np.asarray(patch_weight, np.float32).reshape(-1)
    e = np.exp(pw - pw.max())
    w = (e / e.sum()).reshape(K, K)
    Hr = H.reshape(B, G, K, HID)
    C = np.einsum('kl,bgkh->bglh', w, Hr)
    Pg = np.einsum('bglh,bmlh->bgm', C, Hr)

    def leaky(v):
        return np.where(v >= 0, v, np.float32(0.01) * v)
    Pg = leaky(Pg * pair_w1 + pair_b1)
    Pg = leaky(Pg * pair_w2 + pair_b2)
    m = Pg.max(-1, keepdims=True)
    ex = np.exp(Pg - m)
    r = ex / ex.sum(-1, keepdims=True)
    rw = (1.0 / (1.0 + r * r)).sum(-1)
    sc = rw @ np.asarray(proj_w, np.float32).T + np.asarray(proj_b, np.float32)
    Hs = H * sc[:, :, None]
    z = leaky(Hs @ np.asarray(down_w, np.float32).T + down_b)
    z = leaky(z @ np.asarray(up_w, np.float32).T + up_b)
    return z.astype(np.float32)


def kernel(x, W, b, patch_weight, pair_w1, pair_b1, pair_w2, pair_b2,
           proj_w, proj_b, down_w, down_b, up_w, up_b):
    args = (x, W, b, patch_weight, pair_w1, pair_b1, pair_w2, pair_b2,
            proj_w, proj_b, down_w, down_b, up_w, up_b)
    try:
        in_maps = _host_prepare(*args)
        return _assemble_output(_device_run(in_maps))
    except Exception:
        import traceback
        traceback.print_exc(file=sys.stderr)
        print("kernel: device path failed; numpy fallback", file=sys.stderr)
        return _cpu_fallback(*args)


# revision 46
# speedup vs baseline: 38.9859x; 38.9859x over previous
"""nn_AutoregBlock forward on 8 trn2 NeuronCores (data-parallel over batch).

Self-contained: inline graph construction (bass/tile), host-side input
arrangement, device execution via run_bass_kernel_spmd, output reassembly.
Falls back to a vectorized numpy implementation if the device path fails.
"""
import os
import sys
import numpy as np

B, DIM, HID, K, G, MID = 1024, 384, 64, 8, 48, 128
EPS = 1e-6
NCORES = 8
BL = B // NCORES
R = 16
NBLK = BL // R
NPAIR = R // 2
NCH = DIM // 128


# --------------------------------------------------------------------------
# graph construction (see kernel_build.py history for design notes)
# --------------------------------------------------------------------------
def _legalize_waits(nc, max_waits=1):
    import concourse.mybir as mybir
    total = 0
    for fn in nc.m.functions:
        for bb in fn.blocks:
            new_insts = []
            for inst in bb.instructions:
                si = inst.sync_info
                waits = list(si.on_wait) if si and si.on_wait else []
                if len(waits) > max_waits:
                    extra, keep = waits[:-max_waits], waits[-max_waits:]
                    for k, w in enumerate(extra):
                        nop = mybir.InstNoOp(name=f"{inst.name}_lw{k}")
                        nop.engine = inst.engine
                        nop.sync_info = mybir.SyncInfo(on_wait=[w], on_update=[])
                        new_insts.append(nop)
                        total += 1
                    si.on_wait = keep
                new_insts.append(inst)
            bb.instructions = new_insts
    return total


def _build_graph(trace_sim=False, act_fn=None):
    import concourse.bass as bass
    import concourse.mybir as mybir
    from concourse.tile import TileContext

    f32 = mybir.dt.float32
    bf16 = mybir.dt.bfloat16
    AF = mybir.ActivationFunctionType
    ALU = mybir.AluOpType
    AX = mybir.AxisListType

    nc = bass.Bass()
    dp = nc.declare_dram_parameter
    # x^T / inv^T, x8-replicated, laid out [c][blk][d][r][i] so the DMA
    # source merges (d, r) into one stride-8 run (3-dim DMA AP cap)
    sT = dp("sT", [NCH * NBLK * 128, R * 8], bf16, isOutput=False)
    iT = dp("iT", [NCH * NBLK * 128, R * 8], bf16, isOutput=False)
    Wd = dp("Wd", [DIM, HID], bf16, isOutput=False)
    Bd = dp("Bd", [DIM, HID], bf16, isOutput=False)
    Mw = dp("Mw", [128, 128], bf16, isOutput=False)      # block-diag w
    I1 = dp("I1", [128, 128], bf16, isOutput=False)      # identity
    pw1 = dp("pw1", [128, G], f32, isOutput=False)       # rows 0-47 & 64-111
    pb1 = dp("pb1", [128, G], f32, isOutput=False)
    pw2 = dp("pw2", [128, G], f32, isOutput=False)
    pb2 = dp("pb2", [128, G], f32, isOutput=False)
    pjt = dp("pjt", [128, NCH * 128], bf16, isOutput=False)  # proj_w^T stacked
    pjb = dp("pjb", [128, NCH], f32, isOutput=False)
    W1s = dp("W1s", [128, MID], bf16, isOutput=False)    # down_w^T stacked 2x
    dbv = dp("dbv", [MID, 1], f32, isOutput=False)
    W2s = dp("W2s", [MID, HID], bf16, isOutput=False)    # up_w^T
    ubv = dp("ubv", [128, 1], f32, isOutput=False)       # up_b stacked 2x
    out = dp("out", [128, (BL // 2) * DIM], f32, isOutput=True)

    with TileContext(nc, trace_sim=trace_sim) as tc:
        with tc.tile_pool(name="const", bufs=1) as cp, \
             tc.tile_pool(name="io", bufs=2) as io, \
             tc.tile_pool(name="ctp", bufs=4) as ctpool, \
             tc.tile_pool(name="htp", bufs=64) as htpool, \
             tc.tile_pool(name="pgsp", bufs=NBLK) as pgpool, \
             tc.tile_pool(name="dram", bufs=2, space="DRAM") as dr:
            # ---- constants -------------------------------------------------
            w_rep = cp.tile([128, NCH, R, HID], bf16)   # W chunk bcast over r
            b_rep = cp.tile([128, NCH, R, HID], bf16)
            for c in range(NCH):
                nc.sync.dma_start(
                    out=w_rep[:, c], in_=Wd[128 * c:128 * (c + 1), None, :]
                    .to_broadcast((128, R, HID)))
                nc.sync.dma_start(
                    out=b_rep[:, c], in_=Bd[128 * c:128 * (c + 1), None, :]
                    .to_broadcast((128, R, HID)))
            mwt = cp.tile([128, 128], bf16)
            nc.sync.dma_start(out=mwt[:], in_=Mw[:])
            i1t = cp.tile([128, 128], bf16)
            nc.sync.dma_start(out=i1t[:], in_=I1[:])
            pw1t = cp.tile([128, G], f32)
            nc.sync.dma_start(out=pw1t[:], in_=pw1[:])
            pb1t = cp.tile([128, G], f32)
            nc.sync.dma_start(out=pb1t[:], in_=pb1[:])
            pw2t = cp.tile([128, G], f32)
            nc.sync.dma_start(out=pw2t[:], in_=pw2[:])
            pb2t = cp.tile([128, G], f32)
            nc.sync.dma_start(out=pb2t[:], in_=pb2[:])
            pjtt = cp.tile([128, NCH * 128], bf16)
            nc.sync.dma_start(out=pjtt[:], in_=pjt[:])
            pjbt = cp.tile([128, NCH], f32)
            nc.sync.dma_start(out=pjbt[:], in_=pjb[:])
            w1t = cp.tile([128, MID], bf16)
            nc.sync.dma_start(out=w1t[:], in_=W1s[:])
            dbt = cp.tile([MID, 1], f32)
            nc.sync.dma_start(out=dbt[:], in_=dbv[:])
            w2t = cp.tile([MID, HID], bf16)
            nc.sync.dma_start(out=w2t[:], in_=W2s[:])
            ubt = cp.tile([128, 1], f32)
            nc.sync.dma_start(out=ubt[:], in_=ubv[:])

            hts = []       # 64 per-pair Ht tiles (later scaled in place)
            pgs = []       # per-block Pg SBUF tiles

            # ================= phase A1 ====================================
            with tc.tile_pool(name="psA", bufs=2, space="PSUM") as pp:
                for blk in range(NBLK):
                    r0 = blk * R
                    s_rep = io.tile([128, NCH, R, 8, 8], bf16, tag="s_rep")
                    i_rep = io.tile([128, NCH, R, 8, 8], bf16, tag="i_rep")
                    for c in range(NCH):
                        row = (c * NBLK + blk) * 128
                        nc.sync.dma_start(
                            out=s_rep[:, c],
                            in_=sT[row:row + 128, :]
                            .rearrange("p (r i) -> p r i", i=8)[:, :, None, :]
                            .to_broadcast((128, R, 8, 8)))
                        nc.sync.dma_start(
                            out=i_rep[:, c],
                            in_=iT[row:row + 128, :]
                            .rearrange("p (r i) -> p r i", i=8)[:, :, None, :]
                            .to_broadcast((128, R, 8, 8)))
                    hd = io.tile([128, NCH, R, HID], bf16, tag="hd")
                    for c in range(NCH):
                        t1 = io.tile([128, R, HID], bf16, tag="t1")
                        nc.vector.tensor_tensor(
                            out=t1[:], in0=w_rep[:, c],
                            in1=s_rep[:, c].rearrange("p r a b -> p r (a b)"),
                            op=ALU.mult)
                        t2 = io.tile([128, R, HID], bf16, tag="t2")
                        nc.vector.tensor_tensor(
                            out=t2[:], in0=b_rep[:, c],
                            in1=i_rep[:, c].rearrange("p r a b -> p r (a b)"),
                            op=ALU.mult)
                        av = io.tile([128, R, HID], bf16, tag="av")
                        nc.vector.tensor_tensor(out=av[:], in0=t1[:], in1=t2[:],
                                                op=ALU.add)
                        nc.scalar.activation(hd[:, c], av[:], AF.Gelu)

                    pgp = pp.tile([128, 512], f32, tag="pg")
                    for p in range(NPAIR):
                        ctp = pp.tile([128, 512], f32, tag="ctp")
                        htp = pp.tile([128, 512], f32, tag="htp")
                        for c in range(NCH):
                            lhs = hd[:, c, 2 * p:2 * p + 2, :]
                            nc.tensor.matmul(ctp[:, 128 * c:128 * (c + 1)],
                                             lhs, mwt[:], start=True, stop=True)
                            nc.tensor.matmul(htp[:, 128 * c:128 * (c + 1)],
                                             lhs, i1t[:], start=True, stop=True)
                        ct = ctpool.tile([128, DIM], bf16, tag="ct")
                        nc.vector.tensor_copy(ct[:], ctp[:, 0:DIM])
                        ht = htpool.tile([128, DIM], bf16, tag="ht")
                        nc.scalar.copy(ht[:], htp[:, 0:DIM])
                        hts.append(ht)
                        ctv = ct[:].rearrange("p (g k) -> p k g", k=K)
                        htv = ht[:].rearrange("p (g k) -> p k g", k=K)
                        for l in range(K):
                            nc.tensor.matmul(
                                pgp[0:G, G * p:G * (p + 1)],
                                ctv[0:64, l], htv[0:64, l],
                                start=(l == 0), stop=(l == K - 1))
                        for l in range(K):
                            nc.tensor.matmul(
                                pgp[64:64 + G, G * p:G * (p + 1)],
                                ctv[64:128, l], htv[64:128, l],
                                start=(l == 0), stop=(l == K - 1),
                                tile_position=(64, 64))
                    pgt = pgpool.tile([112, NPAIR * G], f32, tag="pgs")
                    nc.vector.tensor_copy(pgt[:], pgp[0:112, 0:NPAIR * G])
                    pgs.append(pgt)

            # ================= phase A2 ====================================
            with tc.tile_pool(name="psS", bufs=2, space="PSUM") as sp:
                for blk in range(NBLK):
                    g1 = pgs[blk][:].rearrange("p (n g) -> p n g", g=G)
                    w1b = pw1t[0:112, None, :].to_broadcast((112, NPAIR, G))
                    b1b = pb1t[0:112, None, :].to_broadcast((112, NPAIR, G))
                    w2b = pw2t[0:112, None, :].to_broadcast((112, NPAIR, G))
                    b2b = pb2t[0:112, None, :].to_broadcast((112, NPAIR, G))
                    nc.vector.tensor_tensor(out=g1, in0=g1, in1=w1b, op=ALU.mult)
                    nc.vector.tensor_tensor(out=g1, in0=g1, in1=b1b, op=ALU.add)
                    nc.scalar.activation(g1, g1, AF.Lrelu, alpha=0.01)
                    nc.vector.tensor_tensor(out=g1, in0=g1, in1=w2b, op=ALU.mult)
                    nc.vector.tensor_tensor(out=g1, in0=g1, in1=b2b, op=ALU.add)
                    nc.scalar.activation(g1, g1, AF.Lrelu, alpha=0.01)
                    mx = io.tile([112, NPAIR], f32, tag="mx")
                    nc.vector.tensor_reduce(mx[:], g1, axis=AX.X, op=ALU.max)
                    nc.vector.tensor_tensor(
                        out=g1, in0=g1,
                        in1=mx[:, :, None].to_broadcast((112, NPAIR, G)),
                        op=ALU.subtract)
                    nc.scalar.activation(g1, g1, AF.Exp)
                    sm = io.tile([112, NPAIR], f32, tag="sm")
                    nc.vector.tensor_reduce(sm[:], g1, axis=AX.X, op=ALU.add)
                    si = io.tile([112, NPAIR], f32, tag="si")
                    nc.vector.reciprocal(si[:], sm[:])
                    si2 = io.tile([112, NPAIR], f32, tag="si2")
                    nc.vector.tensor_tensor(out=si2[:], in0=si[:], in1=si[:],
                                            op=ALU.mult)
                    nc.scalar.activation(g1, g1, AF.Square)
                    nc.vector.tensor_tensor(
                        out=g1, in0=g1,
                        in1=si2[:, :, None].to_broadcast((112, NPAIR, G)),
                        op=ALU.mult)
                    # 1/(1+q) = exp(-ln(q+1))
                    nc.scalar.activation(g1, g1, AF.Ln, bias=1.0)
                    nc.scalar.activation(g1, g1, AF.Exp, scale=-1.0)
                    rw = io.tile([112, NPAIR], f32, tag="rw")
                    nc.vector.tensor_reduce(rw[:], g1, axis=AX.X, op=ALU.add)
                    # center: bf16 ulp at ~47.7 is 0.125; fold +47 into pjb
                    rwb = io.tile([112, NPAIR], bf16, tag="rwb")
                    nc.vector.tensor_scalar_add(rwb[:], rw[:], -47.0)

                    scp = sp.tile([128, 512], f32, tag="scp")
                    scpv = scp[:, 0:NCH * 2 * NPAIR].rearrange(
                        "p (c r n) -> p c r n", c=NCH, r=2)
                    for c in range(NCH):
                        nc.tensor.matmul(scpv[:, c, 0],
                                         pjtt[0:G, 128 * c:128 * (c + 1)],
                                         rwb[0:G, :], start=True, stop=True)
                        nc.tensor.matmul(scpv[:, c, 1],
                                         pjtt[64:64 + G, 128 * c:128 * (c + 1)],
                                         rwb[64:64 + G, :], start=True, stop=True,
                                         tile_position=(64, 0))
                    scs = io.tile([128, 2, NCH, NPAIR], bf16, tag="scs")
                    for c in range(NCH):
                        nc.scalar.activation(scs[:, :, c, :], scpv[:, c],
                                             AF.Identity, bias=pjbt[:, c:c + 1])
                    dsc = dr.tile([2, NPAIR, NCH, 128], bf16, tag="dsc")
                    scb = io.tile([128, NPAIR, DIM], bf16, tag="scb")
                    # SBUF (128=d, rhat, c, pair) -> DRAM [rhat][p][c][d],
                    # then one contiguous h-broadcast read per parity
                    for rh in range(2):
                        for c in range(NCH):
                            nc.sync.dma_start(
                                out=dsc[rh, :, c, :].rearrange("p d -> d p"),
                                in_=scs[:, rh, c, :])
                        nc.sync.dma_start(
                            out=scb[64 * rh:64 * (rh + 1)],
                            in_=dsc[rh].rearrange("p c d -> (p c d)")
                            [None].to_broadcast((64, NPAIR * DIM))
                            .rearrange("h (p d) -> h p d", p=NPAIR))
                    for p in range(NPAIR):
                        ht = hts[blk * NPAIR + p]
                        nc.vector.tensor_tensor(out=ht[:], in0=ht[:],
                                                in1=scb[:, p, :], op=ALU.mult)

            # ================= phase B =====================================
            with tc.tile_pool(name="psB", bufs=2, space="PSUM") as bp:
                for p2 in range(BL // 4):           # 2 pairs per group
                    z2p = bp.tile([128, 2, 512], f32, tag="z2p")
                    for pi in range(2):
                        hs = hts[2 * p2 + pi]
                        z1p = bp.tile([MID, 2, 512], f32, tag="z1p")
                        nc.tensor.matmul(z1p[:, 0, 0:DIM], w1t[0:64, :],
                                         hs[0:64, :], start=True, stop=True)
                        nc.tensor.matmul(z1p[:, 1, 0:DIM], w1t[64:128, :],
                                         hs[64:128, :], start=True,
                                         stop=True, tile_position=(64, 0))
                        z1s = io.tile([MID, 2, DIM], bf16, tag="z1s")
                        nc.scalar.activation(z1s[:], z1p[:, :, 0:DIM], AF.Lrelu,
                                             bias=dbt[:], alpha=0.01)
                        nc.tensor.matmul(z2p[0:64, pi, 0:DIM], w2t[:],
                                         z1s[:, 0], start=True, stop=True)
                        nc.tensor.matmul(z2p[64:128, pi, 0:DIM], w2t[:],
                                         z1s[:, 1], start=True, stop=True,
                                         tile_position=(0, 64))
                    z2s = io.tile([128, 2, DIM], f32, tag="z2s")
                    nc.scalar.activation(z2s[:], z2p[:, :, 0:DIM], AF.Lrelu,
                                         bias=ubt[:], alpha=0.01)
                    nc.sync.dma_start(
                        out=out[:, (2 * p2) * DIM:(2 * p2 + 2) * DIM],
                        in_=z2s[:])
    _legalize_waits(nc)
    return nc



def _host_prepare(x, W, b, patch_weight, pair_w1, pair_b1, pair_w2, pair_b2,
                 proj_w, proj_b, down_w, down_b, up_w, up_b):
    """Build the per-core input maps (numpy only)."""
    import ml_dtypes
    bf = ml_dtypes.bfloat16
    x = np.asarray(x, np.float32)
    inv = 1.0 / (np.abs(x) + np.float32(EPS))
    pw = np.asarray(patch_weight, np.float32).reshape(-1)
    e = np.exp(pw - pw.max())
    w = (e / e.sum()).reshape(K, K)
    mw = np.zeros((128, 128), np.float32)
    for g in range(16):
        mw[g * K:(g + 1) * K, g * K:(g + 1) * K] = w       # Mw[8g+k,8g+l]=w[k,l]
    stack = lambda a: np.concatenate([a, a], axis=0)

    def gate(m):
        t = np.zeros((128, G), np.float32)
        t[0:G] = m
        t[64:64 + G] = m
        return t

    pjt = np.zeros((128, NCH * 128), np.float32)
    for c in range(NCH):
        blkw = np.asarray(proj_w, np.float32)[128 * c:128 * (c + 1), :].T  # (G,128)
        pjt[0:G, 128 * c:128 * (c + 1)] = blkw
        pjt[64:64 + G, 128 * c:128 * (c + 1)] = blkw
    pjb_full = (np.asarray(proj_b, np.float32)
                + 47.0 * np.asarray(proj_w, np.float32).sum(axis=1))
    pjb = pjb_full.reshape(NCH, 128).T.copy()              # (128, NCH)
    w1s = stack(np.asarray(down_w, np.float32).T)          # (128, MID)
    w2s = np.ascontiguousarray(np.asarray(up_w, np.float32).T)  # (MID, HID)
    ub2 = stack(np.asarray(up_b, np.float32).reshape(HID, 1))

    shared = {
        "Wd": np.asarray(W, np.float32).astype(bf),
        "Bd": np.asarray(b, np.float32).astype(bf),
        "Mw": mw.astype(bf),
        "I1": np.eye(128, dtype=np.float32).astype(bf),
        "pw1": gate(np.asarray(pair_w1, np.float32)),
        "pb1": gate(np.asarray(pair_b1, np.float32)),
        "pw2": gate(np.asarray(pair_w2, np.float32)),
        "pb2": gate(np.asarray(pair_b2, np.float32)),
        "pjt": pjt.astype(bf),
        "pjb": np.ascontiguousarray(pjb),
        "W1s": w1s.astype(bf),
        "dbv": np.asarray(down_b, np.float32).reshape(MID, 1),
        "W2s": w2s.astype(bf),
        "ubv": ub2,
    }
    def arrange_rep(a):
        # (BL, DIM) slice -> [c][blk][d][r][i] with x8 replication on i
        t = a.T.reshape(NCH, 128, NBLK, R)            # (c, d, blk, r)
        t = np.repeat(t.transpose(0, 2, 1, 3)[..., None], 8, axis=4)
        return np.ascontiguousarray(
            t.reshape(NCH * NBLK * 128, R * 8)).astype(bf)

    in_maps = []
    for c in range(NCORES):
        m = dict(shared)
        m["sT"] = arrange_rep(x[c * BL:(c + 1) * BL])
        m["iT"] = arrange_rep(inv[c * BL:(c + 1) * BL])
        in_maps.append(m)
    return in_maps



def _assemble_output(results):
    z = np.empty((B, DIM, HID), np.float32)
    for c in range(NCORES):
        arr = results[c]["out"].reshape(2, 64, BL // 2, DIM)
        z[c * BL:(c + 1) * BL] = arr.transpose(2, 0, 3, 1).reshape(BL, DIM, HID)
    return z


_CACHE = {}


def _device_run(in_maps):
    from concourse.bass_utils import run_bass_kernel_spmd
    if "nc" not in _CACHE:
        _CACHE["nc"] = _build_graph()
    res = run_bass_kernel_spmd(_CACHE["nc"], in_maps, list(range(NCORES)))
    return res.results


def _cpu_fallback(x, W, b, patch_weight, pair_w1, pair_b1, pair_w2, pair_b2,
                  proj_w, proj_b, down_w, down_b, up_w, up_b):
    from scipy.special import erf
    x = np.asarray(x, np.float32)
    s = x[:, :, None]
    inv = 1.0 / (np.abs(s) + np.float32(EPS))
    A = s * np.asarray(W, np.float32)[None] + inv * np.asarray(b, np.float32)[None]
    H = 0.5 * A * (1.0 + erf(A / np.sqrt(2.0, dtype=np.float32)))
    pw = np.asarray(patch_weight, np.float32).reshape(-1)
    e = np.exp(pw - pw.max())
    w = (e / e.sum()).reshape(K, K)
    Hr = H.reshape(B, G, K, HID)
    C = np.einsum('kl,bgkh->bglh', w, Hr)
    Pg = np.einsum('bglh,bmlh->bgm', C, Hr)

    def leaky(v):
        return np.where(v >= 0, v, np.float32(0.01) * v)
    Pg = leaky(Pg * pair_w1 + pair_b1)
    Pg = leaky(Pg * pair_w2 + pair_b2)
    m = Pg.max(-1, keepdims=True)
    ex = np.exp(Pg - m)
    r = ex / ex.sum(-1, keepdims=True)
    rw = (1.0 / (1.0 + r * r)).sum(-1)
    sc = rw @ np.asarray(proj_w, np.float32).T + np.asarray(proj_b, np.float32)
    Hs = H * sc[:, :, None]
    z = leaky(Hs @ np.asarray(down_w, np.float32).T + down_b)
    z = leaky(z @ np.asarray(up_w, np.float32).T + up_b)
    return z.astype(np.float32)


def kernel(x, W, b, patch_weight, pair_w1, pair_b1, pair_w2, pair_b2,
           proj_w, proj_b, down_w, down_b, up_w, up_b):
    args = (x, W, b, patch_weight, pair_w1, pair_b1, pair_w2, pair_b2,
            proj_w, proj_b, down_w, down_b, up_w, up_b)
    try:
        in_maps = _host_prepare(*args)
        return _assemble_output(_device_run(in_maps))
    except Exception:
        import traceback
        traceback.print_exc(file=sys.stderr)
        print("kernel: device path failed; numpy fallback", file=sys.stderr)
        return _cpu_fallback(*args)
